# revision 16
# baseline (speedup 1.0000x reference)
"""Trainium2 Bass kernel for nn_Attention_57080115364834.

Reference computation (B=4, C=512, H=W=64, N=H*W=4096 tokens):
    t = x.reshape(b, c, n).swapaxes(1, 2)          # (b, n, c)
    q, k, v = t@Wq.T+bq, t@Wk.T+bk, t@Wv.T+bv
    attn = softmax(q @ k.T / sqrt(c))              # (b, n, n)
    out = (attn @ v) @ Wo.T + bo                   # (b, n, c)
    return out.reshape(b, c, h, w)                 # raw view, no permute

Sharding: 8 cores = 4 batches x 2 query-halves, no collectives.

Host-side algebra removes BOTH weight applications from the key/value
token streams so no projection ever runs over the full 4096-token axis:
  - scores = (t Wq^T)(t Wk^T)^T = t A t^T with A = Wq^T Wk precomputed
    on host.  The device projects only the queries (q' = t A) and uses
    RAW x as the key matrix - the whole K projection disappears.
  - (attn @ v) @ Wo^T = (attn @ t) @ (Wo Wv)^T.  The device contracts
    P against raw x (U = P t, same cost as P @ v), then projects
    U Wvo^T over the core's 2048 queries only - half the cost of
    projecting v over all 4096 tokens, and no work is duplicated
    between the two cores sharing a batch.
  - bk shifts every score in a row n by the same amount (q_n . bk), so
    softmax cancels it exactly: dropped.  bq contributes scale*(bq Wk
    t^T), a per-key row precomputed on host and applied through the
    (otherwise free) bias operand of the Exp activation.  bv/bo fold to
    bo' = Wo bv + bo, applied via K=1 matmuls only when nonzero (the
    compiled variant is keyed on that flag).

Per-core dataflow (matmuls bf16 with f32 PSUM; normalization in f32):
  q'T[c,n]  = A^T-chunks @ tC-chunk     per 512-query chunk (16 MMs)
  ST[m,n]   = tC-chunks @ q'T           (scores, keys = raw x)
  P[m,n]    = exp(ST*scale + sbias)     ScalarE
  acc      += P                         DVE (rowsum accumulate)
  UT[c',n] += xN-chunk.T @ P            PSUM-accumulated over m-tiles
  u[c',n]   = UT evac (bf16, ScalarE)   - no rowsum dependency
  OT[c,n]   = WvoT-chunks @ u           (16 MMs)
  rowsum    = ones.T @ acc (f32r MM); broadcast via K=1 MM;
              rinv = reciprocal_approx_fast (DVE, 128-wide)
  outT[c,n] = OT * rinv                 (DVE, PSUM->SBUF) -> DMA

The previous chunk's rowsum chain and U-projection are emitted between
the next chunk's q'-projection and its scores loop, so the PE never
waits on the ACT/DVE tail.  Out-matmuls trail the scores loop by two
m-tiles to stay clear of the Exp evacuations.
"""

import sys

for _p in ("/opt/trn_rl_repo", "/root/.axon_site/_ro/trn_rl_repo"):
    if _p not in sys.path:
        sys.path.append(_p)

import numpy as np
import ml_dtypes

import concourse.bacc as bacc
import concourse.mybir as mybir
import concourse.tile as tile
from concourse.bass_utils import run_bass_kernel_spmd

DT = mybir.dt.float32
FR = mybir.dt.float32r
BF = mybir.dt.bfloat16
AFT = mybir.ActivationFunctionType
F8 = mybir.dt.float8e4
DR = mybir.MatmulPerfMode.DoubleRow

B, C, HW = 4, 512, 4096          # batch, channels, tokens per batch
NQ = HW // 2                     # q tokens per core (2048)
CK = C // 128                    # contraction chunks (4)
MT = HW // 128                   # key tiles (32)
NB = NQ // 512                   # q-chunks per core (4)
SCALE = 1.0 / float(np.sqrt(C))
N_CORES = 8

_compiled = {}
_ONES = np.ones(128, dtype=np.float32)


def _build(has_bop):
    qoff = 0  # h=1 cores get a host-side token rotation instead (softmax
    # and U = P@t are invariant to a consistent key permutation)
    nc = bacc.Bacc("TRN2", target_bir_lowering=False)

    xt_e = nc.declare_dram_parameter("xt", [C, HW], BF, isOutput=False)
    xn_e = nc.declare_dram_parameter("xn", [128, MT * C], BF, isOutput=False)
    k8a_e = nc.declare_dram_parameter("k8a", [128, 2 * HW], F8, isOutput=False)
    k8b_e = nc.declare_dram_parameter("k8b", [128, 2 * HW], F8, isOutput=False)
    at_e = nc.declare_dram_parameter("at", [C, C], BF, isOutput=False)
    wvot_e = nc.declare_dram_parameter("wvot", [C, C], BF, isOutput=False)
    sbias_e = nc.declare_dram_parameter("sbias", [128, MT], DT, isOutput=False)
    ones_fr_e = nc.declare_dram_parameter("ones_fr", [128], FR, isOutput=False)
    if has_bop:
        bop_e = nc.declare_dram_parameter("bop", [C], FR, isOutput=False)
    out_e = nc.declare_dram_parameter("outT", [C, NQ], DT, isOutput=True)

    with tile.TileContext(nc) as tc:
        with (
            tc.tile_pool(name="tc", bufs=1) as tc_pool,
            tc.tile_pool(name="xn", bufs=1) as xn_pool,
            tc.tile_pool(name="wt", bufs=1) as w_pool,
            tc.tile_pool(name="consts", bufs=1) as c_pool,
            tc.tile_pool(name="qcp", bufs=2) as qc_pool,
            tc.tile_pool(name="pexp", bufs=6) as pe_pool,
            tc.tile_pool(name="accp", bufs=2) as acc_pool,
            tc.tile_pool(name="up", bufs=2) as u_pool,
            tc.tile_pool(name="rinvp", bufs=2) as rinv_pool,
            tc.tile_pool(name="srp", bufs=2) as sr_pool,
            tc.tile_pool(name="outp", bufs=5) as oc_pool,
            tc.tile_pool(name="psg", bufs=4, space="PSUM") as ps_gen,
            tc.tile_pool(name="psu", bufs=1, space="PSUM") as ps_ut,
        ):
            tc_sb = [tc_pool.tile([128, NQ], BF, tag=f"t{i}", name=f"t{i}") for i in range(CK)]
            kt8 = [tc_pool.tile([128, 2, HW], F8, tag=f"k8p{p}", name=f"k8p{p}") for p in range(2)]
            xn_sb = xn_pool.tile([128, MT, C], BF, tag="xnb", name="xnb")
            at_sb = [w_pool.tile([128, C], BF, tag=f"a{i}", name=f"a{i}") for i in range(CK)]
            wv_sb = [w_pool.tile([128, C], BF, tag=f"w{i}", name=f"w{i}") for i in range(CK)]
            sbias_t = c_pool.tile([128, MT], DT, tag="sb", name="sbias_t")
            ones_col_r = c_pool.tile([128, 1], FR, tag="onescr", name="ones_col_r")
            ones_row_r = c_pool.tile([1, 128], FR, tag="onesrr", name="ones_row_r")
            if has_bop:
                bop_row = c_pool.tile([1, C], FR, tag="bop", name="bop_row")

            # ---- DMA issue order == consumption order.  All transfers are
            # long-line (>=2KB per partition row) and ride the sync queue so
            # descriptor issue never paces the stream. ----
            k8_es = [k8a_e, k8b_e]

            def kt8_dma(h):  # token half h: columns h*2048..(h+1)*2048 of each j
                for p in range(2):
                    for j in range(2):
                        nc.sync.dma_start(
                            kt8[p][:, j, h * 2048:(h + 1) * 2048],
                            k8_es[p][:, j * HW + h * 2048:j * HW + (h + 1) * 2048],
                        )

            def xn_dma(q):  # 4-mt piece q
                nc.sync.dma_start(
                    xn_sb[:, 4 * q:4 * (q + 1), :],
                    xn_e[:, 4 * q * C:4 * (q + 1) * C],
                )

            for i in range(CK):
                nc.sync.dma_start(at_sb[i][:], at_e[i * 128:(i + 1) * 128, :])
                nc.sync.dma_start(
                    tc_sb[i][:, 0:1024], xt_e[i * 128:(i + 1) * 128, qoff:qoff + 1024]
                )
            kt8_dma(0)
            xn_dma(0)
            xn_dma(1)
            kt8_dma(1)
            xn_dma(2)
            xn_dma(3)
            nc.sync.dma_start(ones_col_r[:, 0:1], ones_fr_e[:])
            nc.sync.dma_start(ones_row_r[0:1, :], ones_fr_e[:])
            nc.sync.dma_start(sbias_t[:], sbias_e[:, :])
            for i in range(CK):
                nc.sync.dma_start(
                    tc_sb[i][:, 1024:2048],
                    xt_e[i * 128:(i + 1) * 128, qoff + 1024:qoff + 2048],
                )
            for q in range(4, 8):
                xn_dma(q)
            for i in range(CK):
                nc.sync.dma_start(wv_sb[i][:], wvot_e[i * 128:(i + 1) * 128, :])
            if has_bop:
                nc.sync.dma_start(bop_row[0:1, :], bop_e[:])

            # ---- HAM warm-up: dummy matmuls on never-written SBUF keep the
            # PE clock-gate busy while the first real DMAs land ----
            warm = c_pool.tile([128, 512], BF, tag="warm", name="warm")
            nc.any.memset(warm[:], 0)
            for _ in range(8):
                wps = ps_gen.tile([128, 512], DT, tag="g", name="wps")
                nc.tensor.matmul(wps[:], warm[:, 0:128], warm[:], start=True, stop=True)

            def emit_qproj(nb):
                q8p = [qc_pool.tile([128, 2, 512], F8, tag=f"q8p{p}", name=f"q8p{p}")
                       for p in range(2)]
                qcs = q8p
                for w in range(2):
                    pqs = []
                    for co in (2 * w, 2 * w + 1):
                        pq = ps_gen.tile([128, 512], DT, tag="g", name="pq")
                        for ci in range(CK):
                            nc.tensor.matmul(
                                pq[:], at_sb[ci][:, co * 128:(co + 1) * 128],
                                tc_sb[ci][:, nb * 512:(nb + 1) * 512],
                                start=(ci == 0), stop=(ci == CK - 1),
                            )
                        pqs.append(pq)
                    for pq, co in zip(pqs, (2 * w, 2 * w + 1)):
                        nc.vector.tensor_copy(q8p[co // 2][:, co % 2, :], pq[:])
                return qcs

            def emit_rs(acc_d, acc_g):
                rs = ps_gen.tile([1, 512], DT, tag="g", name="rs")
                nc.tensor.matmul(rs[:], ones_col_r[:, 0:1], acc_d[:], start=True, stop=False)
                nc.tensor.matmul(rs[:], ones_col_r[:, 0:1], acc_g[:], start=False, stop=True)
                rs_row = sr_pool.tile([1, 512], FR, tag="rsrow", name="rs_row")
                nc.scalar.activation(rs_row[:], rs[:], AFT.Copy)
                return rs_row

            def emit_rbc(rs_row, rbc=None):
                if rbc is None:
                    rbc = ps_gen.tile([128, 512], DT, tag="g", name="rbc")
                nc.tensor.matmul(rbc[:], ones_row_r[0:1, :], rs_row[0:1, :],
                                 start=True, stop=True)
                rinv = rinv_pool.tile([128, 512], DT, tag="rinv", name="rinv")
                nc.vector.reciprocal_approx_fast(out=rinv[:], in_=rbc[:])
                return rinv

            def emit_store(tnb, ot, co, rinv, eng=None):
                oc = oc_pool.tile([128, 512], DT, tag="oc", name="oc", bufs=5)
                (eng or nc.vector).tensor_mul(oc[:], ot[:], rinv[:])
                nc.sync.dma_start(
                    out_e[co * 128:(co + 1) * 128, tnb * 512:(tnb + 1) * 512], oc[:]
                )

            def emit_tail(tnb, acc_d, acc_g, u_sbs):
                # mid-chunk tail: co-outer U-projection out of the shared
                # pool; the rbc matmul hides behind co=0's MM group so the
                # PE never waits on the ACT rs_row copy
                rs_row = emit_rs(acc_d, acc_g)
                rbc = ps_gen.tile([128, 512], DT, tag="g", name="rbc")
                rinv = None
                for co in range(CK):
                    ot = ps_gen.tile([128, 512], DT, tag="g", name="ot")
                    for ci in range(CK):
                        nc.tensor.matmul(
                            ot[:], wv_sb[ci][:, co * 128:(co + 1) * 128],
                            u_sbs[ci][:],
                            start=(ci == 0),
                            stop=(ci == CK - 1) and not has_bop,
                        )
                    if has_bop:
                        nc.tensor.matmul(
                            ot[:], bop_row[0:1, co * 128:(co + 1) * 128],
                            rs_row[0:1, :], start=False, stop=True,
                            skip_group_check=True,
                        )
                    if co == 0:
                        rinv = emit_rbc(rs_row, rbc)
                    emit_store(tnb, ot, co, rinv)

            def emit_final_tail(tnb, acc_d, acc_g, u_sbs):
                # ci-outer so the PE restarts right after the first U-chunk
                # evacuation; rowsum chain interleaved between MM groups;
                # OT reuses the UT banks as their evacuations complete.
                ots = [ps_ut.tile([128, 512], DT, tag=f"ut{co}", name="otf")
                       for co in range(CK)]
                for ci in range(CK):
                    for co in range(CK):
                        nc.tensor.matmul(
                            ots[co][:], wv_sb[ci][:, co * 128:(co + 1) * 128],
                            u_sbs[ci][:],
                            start=(ci == 0),
                            stop=(ci == CK - 1) and not has_bop,
                            skip_group_check=True,
                        )
                    if ci == 0:
                        rs_row = emit_rs(acc_d, acc_g)
                    if ci == 1:
                        rinv = emit_rbc(rs_row)
                for co in range(CK):
                    if has_bop:
                        nc.tensor.matmul(
                            ots[co][:], bop_row[0:1, co * 128:(co + 1) * 128],
                            rs_row[0:1, :], start=False, stop=True,
                            skip_group_check=True,
                        )
                    emit_store(tnb, ots[co], co, rinv)

            prev = None
            for nb in range(NB):
                qcs = emit_qproj(nb)
                if prev is not None:
                    emit_tail(*prev)

                acc_d = acc_pool.tile([128, 512], FR, tag="accd", name="accd")
                acc_g = acc_pool.tile([128, 512], FR, tag="accg", name="accg")
                uts = [ps_ut.tile([128, 512], DT, tag=f"ut{co}", name=f"ut{co}") for co in range(CK)]
                pexps = {}

                def emit_out(m):
                    pe = pexps.pop(m)
                    for co in range(CK):
                        nc.tensor.matmul(
                            uts[co][:], xn_sb[:, m, co * 128:(co + 1) * 128],
                            pe[:], start=(m == 0), stop=(m == MT - 1),
                            skip_group_check=True,
                        )

                for mt in range(MT):
                    st = ps_gen.tile([128, 512], DT, tag="g", name="st")
                    for p in range(2):
                        nc.tensor.matmul(
                            st[:], kt8[p][:, :, mt * 128:(mt + 1) * 128],
                            qcs[p][:, :, :], start=(p == 0), stop=(p == 1),
                            perf_mode=DR,
                        )
                    pe = pe_pool.tile([128, 512], BF, tag="pe", name="pexp")
                    nc.scalar.activation(pe[:], st[:], AFT.Exp,
                                         bias=sbias_t[:, mt:mt + 1], scale=SCALE)
                    # rowsum accumulation split across DVE and GpSimd so the
                    # DVE can release the q'-projection banks promptly
                    if mt % 2 == 0:
                        if mt == 0:
                            nc.vector.tensor_copy(acc_d[:], pe[:])
                        else:
                            nc.vector.tensor_add(acc_d[:], acc_d[:], pe[:])
                    else:
                        if mt == 1:
                            nc.gpsimd.tensor_copy(acc_g[:], pe[:])
                        else:
                            nc.gpsimd.tensor_add(acc_g[:], acc_g[:], pe[:])
                    pexps[mt] = pe
                    if mt >= 2:
                        emit_out(mt - 2)
                emit_out(MT - 2)
                emit_out(MT - 1)

                final = nb == NB - 1
                u_sbs = []
                for ci in range(CK):
                    u = u_pool.tile([128, 512], BF, tag=f"u{ci}", name=f"u{ci}")
                    if final and ci >= 2:
                        nc.vector.tensor_copy(u[:], uts[ci][:])
                    else:
                        nc.scalar.activation(u[:], uts[ci][:], AFT.Copy)
                    u_sbs.append(u)
                prev = (nb, acc_d, acc_g, u_sbs)

            emit_final_tail(*prev)

    nc.compile()
    return nc


def _get_compiled(has_bop=False):
    if has_bop not in _compiled:
        _compiled[has_bop] = _build(has_bop)
    return _compiled[has_bop]


def kernel(**inputs):
    x = np.ascontiguousarray(np.asarray(inputs["x"], dtype=np.float32))
    wq = np.asarray(inputs["Wq"], dtype=np.float32)
    wk = np.asarray(inputs["Wk"], dtype=np.float32)
    wv = np.asarray(inputs["Wv"], dtype=np.float32)
    wo = np.asarray(inputs["Wo"], dtype=np.float32)
    bq = np.asarray(inputs["bq"], dtype=np.float32)
    bv = np.asarray(inputs["bv"], dtype=np.float32)
    bo = np.asarray(inputs["bo"], dtype=np.float32)

    at = np.ascontiguousarray((wq.T @ wk).astype(ml_dtypes.bfloat16))
    wvot = np.ascontiguousarray((wo @ wv).T.astype(ml_dtypes.bfloat16))
    bop = wo @ bv + bo
    has_bop = bool(np.any(bop != 0.0))
    bop_fr = np.ascontiguousarray(bop.astype(np.float32))

    xb = x.reshape(B, C, HW)
    xt_bf = xb.astype(ml_dtypes.bfloat16)
    x8 = xb.astype(ml_dtypes.float8_e4m3fn)
    # per-key score bias from bq (zero when bq == 0), pre-scaled
    rrow = (SCALE * ((bq @ wk) @ xb)).astype(np.float32)  # (B, HW)

    in_maps = []
    for core in range(N_CORES):
        bi, h = core // 2, core % 2
        if h == 0:
            xt_c, x8_c, r_c = xt_bf[bi], x8[bi], rrow[bi]
        else:
            # rotate the token axis so this core's queries sit at offset 0;
            # key order is consistently permuted everywhere (softmax and
            # U = P@t are invariant to that)
            xt_c = np.concatenate([xt_bf[bi][:, NQ:], xt_bf[bi][:, :NQ]], axis=1)
            x8_c = np.concatenate([x8[bi][:, NQ:], x8[bi][:, :NQ]], axis=1)
            r_c = np.concatenate([rrow[bi][NQ:], rrow[bi][:NQ]])
        k8p = x8_c.reshape(2, 2, 128, HW)
        m = {
            "xt": np.ascontiguousarray(xt_c),
            "xn": np.ascontiguousarray(xt_c.T.reshape(MT, 128, C).swapaxes(0, 1)
                                       .reshape(128, MT * C)),
            "k8a": np.ascontiguousarray(k8p[0].swapaxes(0, 1).reshape(128, 2 * HW)),
            "k8b": np.ascontiguousarray(k8p[1].swapaxes(0, 1).reshape(128, 2 * HW)),
            "at": at, "wvot": wvot,
            "sbias": np.ascontiguousarray(r_c.reshape(MT, 128).T),
            "ones_fr": _ONES,
        }
        if has_bop:
            m["bop"] = bop_fr
        in_maps.append(m)

    nc = _get_compiled(has_bop)
    res = run_bass_kernel_spmd(nc, in_maps, core_ids=list(range(N_CORES)))

    out = np.empty((B, HW, C), dtype=np.float32)
    for core in range(N_CORES):
        bi, h = core // 2, core % 2
        out[bi, h * NQ:(h + 1) * NQ, :] = res.results[core]["outT"].T
    return out.reshape(B, C, 64, 64)


# revision 18
# speedup vs baseline: 1.0232x; 1.0232x over previous
"""Trainium2 Bass kernel for nn_Attention_57080115364834.

Reference computation (B=4, C=512, H=W=64, N=H*W=4096 tokens):
    t = x.reshape(b, c, n).swapaxes(1, 2)          # (b, n, c)
    q, k, v = t@Wq.T+bq, t@Wk.T+bk, t@Wv.T+bv
    attn = softmax(q @ k.T / sqrt(c))              # (b, n, n)
    out = (attn @ v) @ Wo.T + bo                   # (b, n, c)
    return out.reshape(b, c, h, w)                 # raw view, no permute

Sharding: 8 cores = 4 batches x 2 query-halves, no collectives.

Host-side algebra removes BOTH weight applications from the key/value
token streams so no projection ever runs over the full 4096-token axis:
  - scores = (t Wq^T)(t Wk^T)^T = t A t^T with A = Wq^T Wk precomputed
    on host.  The device projects only the queries (q' = t A) and uses
    RAW x as the key matrix - the whole K projection disappears.
  - (attn @ v) @ Wo^T = (attn @ t) @ (Wo Wv)^T.  The device contracts
    P against raw x (U = P t, same cost as P @ v), then projects
    U Wvo^T over the core's 2048 queries only - half the cost of
    projecting v over all 4096 tokens, and no work is duplicated
    between the two cores sharing a batch.
  - bk shifts every score in a row n by the same amount (q_n . bk), so
    softmax cancels it exactly: dropped.  bq contributes scale*(bq Wk
    t^T), a per-key row precomputed on host and applied through the
    (otherwise free) bias operand of the Exp activation.  bv/bo fold to
    bo' = Wo bv + bo, applied via K=1 matmuls only when nonzero (the
    compiled variant is keyed on that flag).

Per-core dataflow (matmuls bf16 with f32 PSUM; normalization in f32):
  q'T[c,n]  = A^T-chunks @ tC-chunk     per 512-query chunk (16 MMs)
  ST[m,n]   = tC-chunks @ q'T           (scores, keys = raw x)
  P[m,n]    = exp(ST*scale + sbias)     ScalarE
  acc      += P                         DVE (rowsum accumulate)
  UT[c',n] += xN-chunk.T @ P            PSUM-accumulated over m-tiles
  u[c',n]   = UT evac (bf16, ScalarE)   - no rowsum dependency
  OT[c,n]   = WvoT-chunks @ u           (16 MMs)
  rowsum    = ones.T @ acc (f32r MM); broadcast via K=1 MM;
              rinv = reciprocal_approx_fast (DVE, 128-wide)
  outT[c,n] = OT * rinv                 (DVE, PSUM->SBUF) -> DMA

The previous chunk's rowsum chain and U-projection are emitted between
the next chunk's q'-projection and its scores loop, so the PE never
waits on the ACT/DVE tail.  Out-matmuls trail the scores loop by two
m-tiles to stay clear of the Exp evacuations.
"""

import sys

for _p in ("/opt/trn_rl_repo", "/root/.axon_site/_ro/trn_rl_repo"):
    if _p not in sys.path:
        sys.path.append(_p)

import numpy as np
import ml_dtypes

import concourse.bacc as bacc
import concourse.mybir as mybir
import concourse.tile as tile
from concourse.bass_utils import run_bass_kernel_spmd

DT = mybir.dt.float32
FR = mybir.dt.float32r
BF = mybir.dt.bfloat16
AFT = mybir.ActivationFunctionType
F8 = mybir.dt.float8e4
DR = mybir.MatmulPerfMode.DoubleRow

B, C, HW = 4, 512, 4096          # batch, channels, tokens per batch
NQ = HW // 2                     # q tokens per core (2048)
CK = C // 128                    # contraction chunks (4)
MT = HW // 128                   # key tiles (32)
NB = NQ // 512                   # q-chunks per core (4)
SCALE = 1.0 / float(np.sqrt(C))
N_CORES = 8

_compiled = {}
_ONES = np.ones(128, dtype=np.float32)


def _build(has_bop):
    qoff = 0  # h=1 cores get a host-side token rotation instead (softmax
    # and U = P@t are invariant to a consistent key permutation)
    nc = bacc.Bacc("TRN2", target_bir_lowering=False)

    xt_e = nc.declare_dram_parameter("xt", [C, HW], BF, isOutput=False)
    xn_e = nc.declare_dram_parameter("xn", [128, MT * C], BF, isOutput=False)
    k8a_e = nc.declare_dram_parameter("k8a", [128, 2 * HW], F8, isOutput=False)
    k8b_e = nc.declare_dram_parameter("k8b", [128, 2 * HW], F8, isOutput=False)
    at_e = nc.declare_dram_parameter("at", [C, C], BF, isOutput=False)
    wvot_e = nc.declare_dram_parameter("wvot", [C, C], BF, isOutput=False)
    sbias_e = nc.declare_dram_parameter("sbias", [128, MT], DT, isOutput=False)
    ones_fr_e = nc.declare_dram_parameter("ones_fr", [128], FR, isOutput=False)
    if has_bop:
        bop_e = nc.declare_dram_parameter("bop", [C], FR, isOutput=False)
    out_e = nc.declare_dram_parameter("outT", [C, NQ], DT, isOutput=True)

    with tile.TileContext(nc) as tc:
        with (
            tc.tile_pool(name="tc", bufs=1) as tc_pool,
            tc.tile_pool(name="xn", bufs=1) as xn_pool,
            tc.tile_pool(name="wt", bufs=1) as w_pool,
            tc.tile_pool(name="consts", bufs=1) as c_pool,
            tc.tile_pool(name="qcp", bufs=2) as qc_pool,
            tc.tile_pool(name="pexp", bufs=6) as pe_pool,
            tc.tile_pool(name="accp", bufs=2) as acc_pool,
            tc.tile_pool(name="up", bufs=2) as u_pool,
            tc.tile_pool(name="rinvp", bufs=2) as rinv_pool,
            tc.tile_pool(name="srp", bufs=2) as sr_pool,
            tc.tile_pool(name="outp", bufs=5) as oc_pool,
            tc.tile_pool(name="psg", bufs=4, space="PSUM") as ps_gen,
            tc.tile_pool(name="psu", bufs=1, space="PSUM") as ps_ut,
        ):
            tc_sb = [tc_pool.tile([128, NQ], BF, tag=f"t{i}", name=f"t{i}") for i in range(CK)]
            kt8 = [tc_pool.tile([128, 2, HW], F8, tag=f"k8p{p}", name=f"k8p{p}") for p in range(2)]
            xn_sb = xn_pool.tile([128, MT, C], BF, tag="xnb", name="xnb")
            at_sb = [w_pool.tile([128, C], BF, tag=f"a{i}", name=f"a{i}") for i in range(CK)]
            wv_sb = [w_pool.tile([128, C], BF, tag=f"w{i}", name=f"w{i}") for i in range(CK)]
            sbias_t = c_pool.tile([128, MT], DT, tag="sb", name="sbias_t")
            ones_col_r = c_pool.tile([128, 1], FR, tag="onescr", name="ones_col_r")
            ones_row_r = c_pool.tile([1, 128], FR, tag="onesrr", name="ones_row_r")
            if has_bop:
                bop_row = c_pool.tile([1, C], FR, tag="bop", name="bop_row")

            # ---- DMA issue order == consumption order, medium-grain
            # (128-256KB) pieces so the 16 DMA queues stay loaded; xn rides
            # the sync queue too (descriptor issue there is ~43ns) ----
            k8_es = [k8a_e, k8b_e]

            def kt8_dma(cg2):  # 1024-token piece cg2 of each (pair, j)
                for p in range(2):
                    for j in range(2):
                        nc.sync.dma_start(
                            kt8[p][:, j, cg2 * 1024:(cg2 + 1) * 1024],
                            k8_es[p][:, j * HW + cg2 * 1024:j * HW + (cg2 + 1) * 1024],
                        )

            def xn_dma(q):  # 2-mt piece q
                nc.sync.dma_start(
                    xn_sb[:, 2 * q:2 * (q + 1), :],
                    xn_e[:, 2 * q * C:2 * (q + 1) * C],
                )

            def tc_dma(j):
                for i in range(CK):
                    nc.sync.dma_start(
                        tc_sb[i][:, j * 512:(j + 1) * 512],
                        xt_e[i * 128:(i + 1) * 128, qoff + j * 512:qoff + (j + 1) * 512],
                    )

            for i in range(CK):
                nc.sync.dma_start(at_sb[i][:, 0:256], at_e[i * 128:(i + 1) * 128, 0:256])
            tc_dma(0)
            kt8_dma(0)
            xn_dma(0)
            xn_dma(1)
            for i in range(CK):
                nc.sync.dma_start(at_sb[i][:, 256:512], at_e[i * 128:(i + 1) * 128, 256:512])
            kt8_dma(1)
            xn_dma(2)
            xn_dma(3)
            nc.sync.dma_start(ones_col_r[:, 0:1], ones_fr_e[:])
            nc.sync.dma_start(ones_row_r[0:1, :], ones_fr_e[:])
            nc.sync.dma_start(sbias_t[:], sbias_e[:, :])
            xn_dma(4)
            xn_dma(5)
            kt8_dma(2)
            xn_dma(6)
            xn_dma(7)
            xn_dma(8)
            xn_dma(9)
            tc_dma(1)
            kt8_dma(3)
            for q in range(10, 16):
                xn_dma(q)
            tc_dma(2)
            tc_dma(3)
            for i in range(CK):
                nc.sync.dma_start(wv_sb[i][:], wvot_e[i * 128:(i + 1) * 128, :])
            if has_bop:
                nc.sync.dma_start(bop_row[0:1, :], bop_e[:])

            # ---- HAM warm-up: dummy matmuls on never-written SBUF keep the
            # PE clock-gate busy while the first real DMAs land ----
            warm = c_pool.tile([128, 512], BF, tag="warm", name="warm")
            nc.any.memset(warm[:], 0)
            for _ in range(8):
                wps = ps_gen.tile([128, 512], DT, tag="g", name="wps")
                nc.tensor.matmul(wps[:], warm[:, 0:128], warm[:], start=True, stop=True)

            def emit_qproj(nb):
                q8p = [qc_pool.tile([128, 2, 512], F8, tag=f"q8p{p}", name=f"q8p{p}")
                       for p in range(2)]
                qcs = q8p
                for w in range(2):
                    pqs = []
                    for co in (2 * w, 2 * w + 1):
                        pq = ps_gen.tile([128, 512], DT, tag="g", name="pq")
                        for ci in range(CK):
                            nc.tensor.matmul(
                                pq[:], at_sb[ci][:, co * 128:(co + 1) * 128],
                                tc_sb[ci][:, nb * 512:(nb + 1) * 512],
                                start=(ci == 0), stop=(ci == CK - 1),
                            )
                        pqs.append(pq)
                    for pq, co in zip(pqs, (2 * w, 2 * w + 1)):
                        nc.vector.tensor_copy(q8p[co // 2][:, co % 2, :], pq[:])
                return qcs

            def emit_rs(acc_d, acc_g):
                rs = ps_gen.tile([1, 512], DT, tag="g", name="rs")
                nc.tensor.matmul(rs[:], ones_col_r[:, 0:1], acc_d[:], start=True, stop=False)
                nc.tensor.matmul(rs[:], ones_col_r[:, 0:1], acc_g[:], start=False, stop=True)
                rs_row = sr_pool.tile([1, 512], FR, tag="rsrow", name="rs_row")
                nc.scalar.activation(rs_row[:], rs[:], AFT.Copy)
                return rs_row

            def emit_rbc(rs_row, rbc=None):
                if rbc is None:
                    rbc = ps_gen.tile([128, 512], DT, tag="g", name="rbc")
                nc.tensor.matmul(rbc[:], ones_row_r[0:1, :], rs_row[0:1, :],
                                 start=True, stop=True)
                rinv = rinv_pool.tile([128, 512], DT, tag="rinv", name="rinv")
                nc.vector.reciprocal_approx_fast(out=rinv[:], in_=rbc[:])
                return rinv

            def emit_store(tnb, ot, co, rinv, eng=None):
                oc = oc_pool.tile([128, 512], DT, tag="oc", name="oc", bufs=5)
                (eng or nc.vector).tensor_mul(oc[:], ot[:], rinv[:])
                nc.sync.dma_start(
                    out_e[co * 128:(co + 1) * 128, tnb * 512:(tnb + 1) * 512], oc[:]
                )

            def emit_tail(tnb, acc_d, acc_g, u_sbs):
                # mid-chunk tail: co-outer U-projection out of the shared
                # pool; the rbc matmul hides behind co=0's MM group so the
                # PE never waits on the ACT rs_row copy
                rs_row = emit_rs(acc_d, acc_g)
                rbc = ps_gen.tile([128, 512], DT, tag="g", name="rbc")
                rinv = None
                for co in range(CK):
                    ot = ps_gen.tile([128, 512], DT, tag="g", name="ot")
                    for ci in range(CK):
                        nc.tensor.matmul(
                            ot[:], wv_sb[ci][:, co * 128:(co + 1) * 128],
                            u_sbs[ci][:],
                            start=(ci == 0),
                            stop=(ci == CK - 1) and not has_bop,
                        )
                    if has_bop:
                        nc.tensor.matmul(
                            ot[:], bop_row[0:1, co * 128:(co + 1) * 128],
                            rs_row[0:1, :], start=False, stop=True,
                            skip_group_check=True,
                        )
                    if co == 0:
                        rinv = emit_rbc(rs_row, rbc)
                    emit_store(tnb, ot, co, rinv)

            def emit_final_tail(tnb, acc_d, acc_g, u_sbs):
                # ci-outer so the PE restarts right after the first U-chunk
                # evacuation; rowsum chain interleaved between MM groups;
                # OT reuses the UT banks as their evacuations complete.
                ots = [ps_ut.tile([128, 512], DT, tag=f"ut{co}", name="otf")
                       for co in range(CK)]
                for ci in range(CK):
                    for co in range(CK):
                        nc.tensor.matmul(
                            ots[co][:], wv_sb[ci][:, co * 128:(co + 1) * 128],
                            u_sbs[ci][:],
                            start=(ci == 0),
                            stop=(ci == CK - 1) and not has_bop,
                            skip_group_check=True,
                        )
                    if ci == 0:
                        rs_row = emit_rs(acc_d, acc_g)
                    if ci == 1:
                        rinv = emit_rbc(rs_row)
                for co in range(CK):
                    if has_bop:
                        nc.tensor.matmul(
                            ots[co][:], bop_row[0:1, co * 128:(co + 1) * 128],
                            rs_row[0:1, :], start=False, stop=True,
                            skip_group_check=True,
                        )
                    emit_store(tnb, ots[co], co, rinv)

            prev = None
            for nb in range(NB):
                qcs = emit_qproj(nb)
                if prev is not None:
                    emit_tail(*prev)

                acc_d = acc_pool.tile([128, 512], FR, tag="accd", name="accd")
                acc_g = acc_pool.tile([128, 512], FR, tag="accg", name="accg")
                uts = [ps_ut.tile([128, 512], DT, tag=f"ut{co}", name=f"ut{co}") for co in range(CK)]
                pexps = {}

                def emit_out(m):
                    pe = pexps.pop(m)
                    for co in range(CK):
                        nc.tensor.matmul(
                            uts[co][:], xn_sb[:, m, co * 128:(co + 1) * 128],
                            pe[:], start=(m == 0), stop=(m == MT - 1),
                            skip_group_check=True,
                        )

                for mt in range(MT):
                    st = ps_gen.tile([128, 512], DT, tag="g", name="st")
                    for p in range(2):
                        nc.tensor.matmul(
                            st[:], kt8[p][:, :, mt * 128:(mt + 1) * 128],
                            qcs[p][:, :, :], start=(p == 0), stop=(p == 1),
                            perf_mode=DR,
                        )
                    pe = pe_pool.tile([128, 512], BF, tag="pe", name="pexp")
                    nc.scalar.activation(pe[:], st[:], AFT.Exp,
                                         bias=sbias_t[:, mt:mt + 1], scale=SCALE)
                    # rowsum accumulation split across DVE and GpSimd so the
                    # DVE can release the q'-projection banks promptly
                    if mt % 2 == 0:
                        if mt == 0:
                            nc.vector.tensor_copy(acc_d[:], pe[:])
                        else:
                            nc.vector.tensor_add(acc_d[:], acc_d[:], pe[:])
                    else:
                        if mt == 1:
                            nc.gpsimd.tensor_copy(acc_g[:], pe[:])
                        else:
                            nc.gpsimd.tensor_add(acc_g[:], acc_g[:], pe[:])
                    pexps[mt] = pe
                    if mt >= 2:
                        emit_out(mt - 2)
                emit_out(MT - 2)
                emit_out(MT - 1)

                final = nb == NB - 1
                u_sbs = []
                for ci in range(CK):
                    u = u_pool.tile([128, 512], BF, tag=f"u{ci}", name=f"u{ci}")
                    if final and ci >= 2:
                        nc.vector.tensor_copy(u[:], uts[ci][:])
                    else:
                        nc.scalar.activation(u[:], uts[ci][:], AFT.Copy)
                    u_sbs.append(u)
                prev = (nb, acc_d, acc_g, u_sbs)

            emit_final_tail(*prev)

    nc.compile()
    return nc


def _get_compiled(has_bop=False):
    if has_bop not in _compiled:
        _compiled[has_bop] = _build(has_bop)
    return _compiled[has_bop]


def kernel(**inputs):
    x = np.ascontiguousarray(np.asarray(inputs["x"], dtype=np.float32))
    wq = np.asarray(inputs["Wq"], dtype=np.float32)
    wk = np.asarray(inputs["Wk"], dtype=np.float32)
    wv = np.asarray(inputs["Wv"], dtype=np.float32)
    wo = np.asarray(inputs["Wo"], dtype=np.float32)
    bq = np.asarray(inputs["bq"], dtype=np.float32)
    bv = np.asarray(inputs["bv"], dtype=np.float32)
    bo = np.asarray(inputs["bo"], dtype=np.float32)

    at = np.ascontiguousarray((wq.T @ wk).astype(ml_dtypes.bfloat16))
    wvot = np.ascontiguousarray((wo @ wv).T.astype(ml_dtypes.bfloat16))
    bop = wo @ bv + bo
    has_bop = bool(np.any(bop != 0.0))
    bop_fr = np.ascontiguousarray(bop.astype(np.float32))

    xb = x.reshape(B, C, HW)
    xt_bf = xb.astype(ml_dtypes.bfloat16)
    x8 = xb.astype(ml_dtypes.float8_e4m3fn)
    # per-key score bias from bq (zero when bq == 0), pre-scaled
    rrow = (SCALE * ((bq @ wk) @ xb)).astype(np.float32)  # (B, HW)

    in_maps = []
    for core in range(N_CORES):
        bi, h = core // 2, core % 2
        if h == 0:
            xt_c, x8_c, r_c = xt_bf[bi], x8[bi], rrow[bi]
        else:
            # rotate the token axis so this core's queries sit at offset 0;
            # key order is consistently permuted everywhere (softmax and
            # U = P@t are invariant to that)
            xt_c = np.concatenate([xt_bf[bi][:, NQ:], xt_bf[bi][:, :NQ]], axis=1)
            x8_c = np.concatenate([x8[bi][:, NQ:], x8[bi][:, :NQ]], axis=1)
            r_c = np.concatenate([rrow[bi][NQ:], rrow[bi][:NQ]])
        k8p = x8_c.reshape(2, 2, 128, HW)
        m = {
            "xt": np.ascontiguousarray(xt_c),
            "xn": np.ascontiguousarray(xt_c.T.reshape(MT, 128, C).swapaxes(0, 1)
                                       .reshape(128, MT * C)),
            "k8a": np.ascontiguousarray(k8p[0].swapaxes(0, 1).reshape(128, 2 * HW)),
            "k8b": np.ascontiguousarray(k8p[1].swapaxes(0, 1).reshape(128, 2 * HW)),
            "at": at, "wvot": wvot,
            "sbias": np.ascontiguousarray(r_c.reshape(MT, 128).T),
            "ones_fr": _ONES,
        }
        if has_bop:
            m["bop"] = bop_fr
        in_maps.append(m)

    nc = _get_compiled(has_bop)
    res = run_bass_kernel_spmd(nc, in_maps, core_ids=list(range(N_CORES)))

    out = np.empty((B, HW, C), dtype=np.float32)
    for core in range(N_CORES):
        bi, h = core // 2, core % 2
        out[bi, h * NQ:(h + 1) * NQ, :] = res.results[core]["outT"].T
    return out.reshape(B, C, 64, 64)
